# revision 25
# baseline (speedup 1.0000x reference)
"""CRLI kernel for trn2, 8 NeuronCores (SPMD, one chip).

Sharding: 8 independent generator scans = (fwd/bwd) x (4 batch tiles of 128).
Cores 0-3: forward direction on batch rows [128c, 128c+128); cores 4-7:
backward direction (host time-reverses inputs) on the same rows.  The program
is identical on all cores; direction / batch tile comes from per-core inputs.

All generator/decoder matmuls are activation-stationary: lhsT = transposed
activations [K=feature, M=batch<=128] in bf16, rhs = transposed weights in
bf16 streaming at N=512.  PSUM accumulates fp32; the c-state path stays fp32
(measured end-to-end bf16 error ~1e-3 << 2e-2 tolerance).

The batch-constant decoder (64 autonomous steps) is replicated on every core
and interleaved into the scan.  The final h = (hf + flip(hb))/2 combine is a
pairwise ReduceScatter; the hidden flip is pre-applied on the host by
permuting the bwd layer-1 weights.  The discriminator (zero-state LSTM cells
== feedforward, f-gate pruned) runs 8-way batch-split; its i/o/g gate chains
are kept on identical partition lanes, with 3 independent s-groups packed at
partition bases 0/32/64 via tile_position col-tiling.
"""

import numpy as np
import ml_dtypes

import concourse.bass as bass
import concourse.bacc as bacc
import concourse.mybir as mybir
import concourse.tile as tile

FP = mybir.dt.float32
FR = mybir.dt.float32r
BF = mybir.dt.bfloat16
AF = mybir.ActivationFunctionType
ALU = mybir.AluOpType

B, S, D, H = 512, 64, 8, 512
GATES = 4 * H          # 2048
NB = GATES // 512      # psum banks per gate set
BT = 128               # batch rows per core
NCORES = 8
NPB = ml_dtypes.bfloat16

# disc: (in, hid) per pruned cell
DISC_SZ = [(8, 32), (32, 16), (16, 8), (8, 16), (16, 32)]


def fr(ap):
    return ap.bitcast(FR)


class _PrechargeRemoteSem:
    """Tile's schedule-time CoreSim is single-core: the partner's
    remote_dma sem increments never arrive, so waits on dec_rsem would
    deadlock the scheduling pass.  Pre-charge that sem in the scheduling
    sim only -- the emitted program is unchanged (sems are cleared by the
    kernel preamble at runtime)."""

    def __init__(self, sem_holder):
        self.sem_holder = sem_holder

    def __enter__(self):
        import concourse.bass_interp as BI
        from concourse.bass import create_sync_update
        self.BI = BI
        self.orig = BI.CoreSim.simulate
        holder = self.sem_holder

        def patched(sim_self, *a, **k):
            sem = holder.get("rsem")
            if sem is not None:
                try:
                    for _ in range(2 * S + 4):
                        sim_self.update_semaphore(create_sync_update(sem, 2))
                except Exception:
                    pass
            return self.orig(sim_self, *a, **k)

        BI.CoreSim.simulate = patched
        return self

    def __exit__(self, *a):
        self.BI.CoreSim.simulate = self.orig


def build_program(n_steps=S, sim_mode=False, no_dec=False, no_disc=False,
                  dec_split=False):
    holder = {}
    with _PrechargeRemoteSem(holder):
        return _build_program(n_steps, sim_mode, no_dec, no_disc, dec_split,
                              holder)


def _build_program(n_steps, sim_mode, no_dec, no_disc, dec_split, holder):
    nc = bacc.Bacc("TRN2", target_bir_lowering=False, debug=False)

    def din(name, shape, d=FP):
        return nc.dram_tensor(name, shape, d, kind="ExternalInput")

    xq = din("xq", [BT, S * D])            # mask * values (direction-ordered)
    xom = din("xom", [BT, S * D])          # 1 - mask
    wih0b = din("wih0b", [40, GATES], BF)  # row0 bias, rows 32-39 Wih0.T
    whh0T = din("whh0T", [128, 4 * GATES], BF)
    wih1T = din("wih1T", [128, 4 * GATES], BF)
    whh1T = din("whh1T", [128, 4 * GATES], BF)
    b1row = din("b1row", [1, GATES], BF)
    goT = din("goT", [128, 4 * D], BF)
    gob = din("gob", [1, D], BF)
    h1i = din("h1i", [BT, H])
    c1i = din("c1i", [BT, H])
    h1iT = din("h1iT", [128, 4 * BT], BF)
    ones_bf = din("ones_bf", [1, 128], BF)
    ident = din("ident", [128, 128])
    # decoder
    if dec_split:
        wdecT = din("wdecT", [128, 8 * GATES // 2], BF)
        wdec_b = din("wdec_b", [1, GATES // 2], BF)
        dmine0 = din("dmine0", [128, 4], BF)   # [c_my0,c_my1,h_my0,h_my1]
        drecv0 = din("drecv0", [128, 4], BF)   # partner halves
        dec_c1 = din("dec_c1", [1, H // 2])
    else:
        wdecT = din("wdecT", [128, 8 * GATES], BF)
        wdec_b = din("wdec_b", [1, GATES], BF)
        dstate0 = din("dstate0", [128, 8], BF)
        dec_c1 = din("dec_c1", [1, H])
    wdoT = din("wdoT", [128, 4 * D], BF)
    wdob = din("wdob", [1, D], BF)
    # end phase
    vT = din("vT", [128, 4 * 64])
    mT = din("mT", [128, 4 * 64])
    fcWT = din("fcWT", [128, 4 * H], BF)
    fcb = din("fcb", [1, H], BF)
    ones_f = din("ones_f", [1, 128], BF)
    dw0 = [din(f"dw0_{g}", [D, 32], BF) for g in range(3)]
    dwg = [[din(f"dw{i}_{g}", [96, DISC_SZ[i][1]], BF) for g in range(3)]
           for i in range(1, 5)]
    dwo3 = din("dwo3", [96, D], BF)
    dbs = [din(f"db{i}", [96, 3]) for i in range(5)]
    dbo = din("dbo", [D, 1])

    o_imp = nc.dram_tensor("o_imp", [64, H], FP, kind="ExternalOutput")
    o_disc = nc.dram_tensor("o_disc", [D, 64 * S], FP, kind="ExternalOutput")
    o_lat = nc.dram_tensor("o_lat", [64, H], FP, kind="ExternalOutput")
    o_rec = nc.dram_tensor("o_rec", [1, S * D], FP, kind="ExternalOutput")

    with tile.TileContext(nc) as tc, \
         tc.tile_pool(name="wpool", bufs=1) as wpool, \
         tc.tile_pool(name="state", bufs=2) as spool, \
         tc.tile_pool(name="small", bufs=1) as small, \
         tc.tile_pool(name="gates", bufs=4, space="PSUM") as pg, \
         tc.tile_pool(name="tpose", bufs=2, space="PSUM") as pt, \
         tc.tile_pool(name="dgates", bufs=2, space="PSUM") as pd, \
         tc.tile_pool(name="dram", bufs=1, space="DRAM") as dram:

        def load(t, pool=wpool):
            st = pool.tile(list(t.shape), t.dtype, tag=f"w_{t.name}",
                           name=f"s_{t.name}")
            nc.sync.dma_start(out=st[:], in_=t[:])
            return st

        s_xq = load(xq)
        s_xom = load(xom)
        s_wih0b = load(wih0b)
        s_whh0T = load(whh0T)
        s_goT = load(goT)
        s_gob = load(gob)
        s_onesb = load(ones_bf)
        s_ident = load(ident)
        s_wih1T = load(wih1T)
        s_whh1T = load(whh1T)
        s_b1row = load(b1row)
        s_wdecT = load(wdecT)
        s_wdec_b = load(wdec_b)
        s_wdoT = load(wdoT)
        s_wdob = load(wdob)
        id1 = s_ident[0:1, 0:1]

        h0 = spool.tile([BT, H], FP, tag="h0")
        c0 = spool.tile([BT, H], FP, tag="c0")
        h1 = spool.tile([BT, H], FP, tag="h1")
        c1 = spool.tile([BT, H], FP, tag="c1")
        h0T = spool.tile([128, 4 * BT], BF, tag="h0T")
        h1T = spool.tile([128, 4 * BT], BF, tag="h1T")
        nc.vector.memset(h0[:], 0.0)
        nc.vector.memset(c0[:], 0.0)
        nc.vector.memset(h0T[:], 0.0)
        nc.sync.dma_start(out=h1[:], in_=h1i[:])
        nc.sync.dma_start(out=c1[:], in_=c1i[:])
        nc.sync.dma_start(out=h1T[:], in_=h1iT[:])

        if dec_split:
            dmine = spool.tile([128, 4], BF, tag="dmine")
            nc.sync.dma_start(out=dmine[:], in_=dmine0[:])
            drecvA = wpool.tile([128, 4], BF)
            drecvB = wpool.tile([128, 4], BF)
            nc.sync.dma_start(out=drecvA[:], in_=drecv0[:])
            dc = spool.tile([1, H // 2], FP, tag="dc")
            nc.sync.dma_start(out=dc[:], in_=dec_c1[:])
            rsem = nc.alloc_semaphore("dec_rsem")
            lsem = nc.alloc_semaphore("dec_lsem")
            if not sim_mode:
                holder["rsem"] = rsem
        else:
            dstate = spool.tile([128, 8], BF, tag="dstate")
            nc.sync.dma_start(out=dstate[:], in_=dstate0[:])
            dc = spool.tile([1, H], FP, tag="dc")
            nc.sync.dma_start(out=dc[:], in_=dec_c1[:])
        youts = wpool.tile([1, S * D], FP)

        def cell_acts(gb, c_prev, h_new, c_new):
            """gb: 4 psum [BT,512] banks in order f,i,g,o."""
            sf = small.tile([BT, 512], FP, tag="sf")
            si = small.tile([BT, 512], FP, tag="si")
            tg = small.tile([BT, 512], FP, tag="tg")
            so = small.tile([BT, 512], FP, tag="so")
            nc.scalar.activation(sf[:], gb[0][:], AF.Sigmoid)
            nc.scalar.activation(si[:], gb[1][:], AF.Sigmoid)
            nc.scalar.activation(tg[:], gb[2][:], AF.Tanh)
            nc.scalar.activation(so[:], gb[3][:], AF.Sigmoid)
            t1 = small.tile([BT, 512], FP, tag="t1")
            t2 = small.tile([BT, 512], FP, tag="t2")
            nc.vector.tensor_mul(t1[:], sf[:], c_prev[:])
            nc.vector.tensor_mul(t2[:], si[:], tg[:])
            nc.vector.tensor_add(c_new[:], t1[:], t2[:])
            th = small.tile([BT, 512], FP, tag="th")
            nc.scalar.activation(th[:], c_new[:], AF.Tanh)
            nc.vector.tensor_mul(h_new[:], so[:], th[:])

        def transpose_to_bf(src, dstT):
            """src [BT,512] fp32 -> dstT [128, 4*BT] bf16 (4 PE transposes)."""
            for k in range(4):
                p = pt.tile([128, BT], FP, tag="tp", name=f"tp{k}")
                nc.tensor.matmul(p[:], src[:, 128 * k:128 * (k + 1)],
                                 s_ident[:], is_transpose=True,
                                 start=True, stop=True)
                nc.scalar.activation(dstT[:, BT * k:BT * (k + 1)], p[:],
                                     AF.Copy)

        for t in range(n_steps):
            # ---- ximp = h1 @ gen_out.W.T + b ----
            xp = pt.tile([BT, D], FP, tag="tp")
            for k in range(4):
                nc.tensor.matmul(xp[:], h1T[:, BT * k:BT * (k + 1)],
                                 s_goT[:, D * k:D * (k + 1)],
                                 start=(k == 0), stop=False)
            nc.tensor.matmul(xp[:], s_onesb[:, 0:BT], s_gob[:],
                             start=False, stop=True)
            # in_f = (1-m)*ximp + m*x
            inf = small.tile([BT, D], FP, tag="inf")
            nc.vector.tensor_mul(inf[:], s_xom[:, D * t:D * (t + 1)], xp[:])
            nc.vector.tensor_add(inf[:], inf[:], s_xq[:, D * t:D * (t + 1)])
            infT = small.tile([40, 128], BF, tag="infT")
            pti = pt.tile([D, 128], FP, tag="tp")
            nc.tensor.matmul(pti[:], inf[:], s_ident[:], is_transpose=True,
                             start=True, stop=True)
            nc.vector.memset(infT[0:32, :], 1.0)
            nc.scalar.activation(infT[32:32 + D, :], pti[:], AF.Copy)

            # ---- layer 0 gates (f,i,g,o banks) ----
            g0b = []
            for bk in range(NB):
                gb = pg.tile([BT, 512], FP, tag="g", name=f"g0_{bk}")
                for k in range(4):
                    nc.tensor.matmul(gb[:], h0T[:, BT * k:BT * (k + 1)],
                                     s_whh0T[:, GATES * k + 512 * bk:
                                             GATES * k + 512 * (bk + 1)],
                                     start=(k == 0), stop=False)
                nc.tensor.matmul(gb[:], infT[:],
                                 s_wih0b[:, 512 * bk:512 * (bk + 1)],
                                 start=False, stop=True)
                g0b.append(gb)
            h0n = spool.tile([BT, H], FP, tag="h0")
            c0n = spool.tile([BT, H], FP, tag="c0")
            cell_acts(g0b, c0, h0n, c0n)
            h0Tn = spool.tile([128, 4 * BT], BF, tag="h0T")
            transpose_to_bf(h0n, h0Tn)

            # ---- layer 1 gates ----
            g1b = []
            for bk in range(NB):
                gb = pg.tile([BT, 512], FP, tag="g", name=f"g1_{bk}")
                for k in range(4):
                    nc.tensor.matmul(gb[:], h1T[:, BT * k:BT * (k + 1)],
                                     s_whh1T[:, GATES * k + 512 * bk:
                                             GATES * k + 512 * (bk + 1)],
                                     start=(k == 0), stop=False)
                for k in range(4):
                    nc.tensor.matmul(gb[:], h0Tn[:, BT * k:BT * (k + 1)],
                                     s_wih1T[:, GATES * k + 512 * bk:
                                             GATES * k + 512 * (bk + 1)],
                                     start=False, stop=False)
                last_g1_mm = nc.tensor.matmul(
                    gb[:], s_onesb[:, 0:128],
                    s_b1row[:, 512 * bk:512 * (bk + 1)],
                    start=False, stop=True)
                g1b.append(gb)
            h1n = spool.tile([BT, H], FP, tag="h1")
            c1n = spool.tile([BT, H], FP, tag="c1")
            cell_acts(g1b, c1, h1n, c1n)
            h1Tn = spool.tile([128, 4 * BT], BF, tag="h1T")
            transpose_to_bf(h1n, h1Tn)

            if no_dec:
                h0, c0, h1, c1, h0T, h1T = h0n, c0n, h1n, c1n, h0Tn, h1Tn
                continue
            if dec_split:
                # ---- decoder step, pair-split (my 1024 gates) ----
                HQ = H // 2
                drd = drecvA if t % 2 == 0 else drecvB
                dwr = drecvB if t % 2 == 0 else drecvA
                # state K-tile order: c_my0,c_my1,c_par0,c_par1,h_my0,...
                kt = [dmine[:, 0:1], dmine[:, 1:2], drd[:, 0:1], drd[:, 1:2],
                      dmine[:, 2:3], dmine[:, 3:4], drd[:, 2:3], drd[:, 3:4]]
                last_gen = None
                dg = []
                drecv_mms = []
                for bk in range(2):
                    gb = pd.tile([1, 512], FP, tag="dg", name=f"dg{bk}")
                    for k in range(8):
                        mm = nc.tensor.matmul(
                            gb[:], kt[k],
                            s_wdecT[:, 1024 * k + 512 * bk:
                                    1024 * k + 512 * (bk + 1)],
                            start=(k == 0), stop=False)
                        if k in (2, 3, 6, 7):
                            drecv_mms.append(mm)
                    nc.tensor.matmul(gb[:], s_onesb[0:1, 0:1],
                                     s_wdec_b[:, 512 * bk:512 * (bk + 1)],
                                     start=False, stop=True)
                    dg.append(gb)
                if t > 0:
                    w = nc.tensor.wait_ge(rsem, 2 * t)
                    tile.add_dep_helper(w.ins, last_g1_mm.ins, sync=True,
                                        reason="dec wait after gen mms")
                    for mm in drecv_mms:
                        tile.add_dep_helper(mm.ins, w.ins, sync=True,
                                            reason="dec drecv gated")
                # acts: bank0 = [f|i], bank1 = [g|o]
                dact = small.tile([1, 4 * HQ], FP, tag="dact")
                nc.scalar.activation(dact[:, 0:2 * HQ], dg[0][:], AF.Sigmoid)
                nc.scalar.activation(dact[:, 2 * HQ:3 * HQ],
                                     dg[1][:, 0:HQ], AF.Tanh)
                nc.scalar.activation(dact[:, 3 * HQ:4 * HQ],
                                     dg[1][:, HQ:2 * HQ], AF.Sigmoid)
                dmix = small.tile([1, 2 * HQ], FP, tag="dmix")
                nc.vector.tensor_mul(dmix[:, 0:HQ], dact[:, 0:HQ], dc[:])
                nc.vector.tensor_mul(dmix[:, HQ:2 * HQ], dact[:, HQ:2 * HQ],
                                     dact[:, 2 * HQ:3 * HQ])
                dcn = spool.tile([1, H // 2], FP, tag="dc")
                nc.vector.tensor_add(dcn[:], dmix[:, 0:HQ], dmix[:, HQ:2 * HQ])
                dhc = small.tile([1, 2 * HQ], FP, tag="dhc")
                nc.scalar.activation(dhc[:, 0:HQ], dcn[:], AF.Tanh)
                nc.vector.tensor_mul(dhc[:, HQ:2 * HQ],
                                     dact[:, 3 * HQ:4 * HQ], dhc[:, 0:HQ])
                dsp = pt.tile([128, 4], FP, tag="tp")
                for j in range(2):
                    nc.tensor.matmul(dsp[:, j:j + 1],
                                     dcn[:, 128 * j:128 * (j + 1)], id1,
                                     is_transpose=True, start=True, stop=True)
                    nc.tensor.matmul(dsp[:, 2 + j:3 + j],
                                     dhc[:, HQ + 128 * j:HQ + 128 * (j + 1)],
                                     id1, is_transpose=True,
                                     start=True, stop=True)
                dminen = spool.tile([128, 4], BF, tag="dmine")
                nc.vector.tensor_copy(dminen[:], dsp[:])
                if sim_mode:
                    nc.vector.tensor_copy(dwr[:], dminen[:])
                    nc.gpsimd.sem_inc(rsem, 2)
                else:
                    nc.gpsimd.remote_dma_broadcast(
                        dwr[:], dminen[:], rsem, lsem,
                        rdests=[None, None, None, None,
                                (0, 4), None, None, None])
                    nc.gpsimd.trigger_dma(1)
                # y_{t-1} from full h after step t-1: dmine (pre-update) +
                # drd (exchange t-1) -- both gated by w
                if t > 0:
                    ykt = [dmine[:, 2:3], dmine[:, 3:4],
                           drd[:, 2:3], drd[:, 3:4]]
                    yp = pd.tile([1, D], FP, tag="dg")
                    for k in range(4):
                        mm = nc.tensor.matmul(yp[:], ykt[k],
                                              s_wdoT[:, D * k:D * (k + 1)],
                                              start=(k == 0), stop=False)
                        if k >= 2:
                            tile.add_dep_helper(mm.ins, w.ins, sync=True,
                                                reason="y drecv gated")
                    nc.tensor.matmul(yp[:], s_onesb[0:1, 0:1], s_wdob[:],
                                     start=False, stop=True)
                    nc.scalar.activation(
                        youts[:, D * (t - 1):D * t], yp[:], AF.Copy)
                h0, c0, h1, c1, h0T, h1T = h0n, c0n, h1n, c1n, h0Tn, h1Tn
                dc, dmine = dcn, dminen
                continue
            # ---- decoder step (replicated, bf16) ----
            # ---- decoder step (replicated, bf16) ----
            dg = []
            for bk in range(NB):
                gb = pd.tile([1, 512], FP, tag="dg", name=f"dg{bk}")
                for k in range(8):
                    nc.tensor.matmul(gb[:], dstate[:, k:k + 1],
                                     s_wdecT[:, GATES * k + 512 * bk:
                                             GATES * k + 512 * (bk + 1)],
                                     start=(k == 0), stop=False)
                nc.tensor.matmul(gb[:], s_onesb[0:1, 0:1],
                                 s_wdec_b[:, 512 * bk:512 * (bk + 1)],
                                 start=False, stop=True)
                dg.append(gb)
            dact = small.tile([1, 4 * H], FP, tag="dact")  # sf|si|tg|so
            nc.scalar.activation(dact[:, 0:H], dg[0][:], AF.Sigmoid)
            nc.scalar.activation(dact[:, H:2 * H], dg[1][:], AF.Sigmoid)
            nc.scalar.activation(dact[:, 2 * H:3 * H], dg[2][:], AF.Tanh)
            nc.scalar.activation(dact[:, 3 * H:4 * H], dg[3][:], AF.Sigmoid)
            dmix = small.tile([1, 2 * H], FP, tag="dmix")  # t1|t2
            nc.vector.tensor_mul(dmix[:, 0:H], dact[:, 0:H], dc[:])
            nc.vector.tensor_mul(dmix[:, H:2 * H], dact[:, H:2 * H],
                                 dact[:, 2 * H:3 * H])
            dcn = spool.tile([1, H], FP, tag="dc")
            nc.vector.tensor_add(dcn[:], dmix[:, 0:H], dmix[:, H:2 * H])
            dhc = small.tile([1, 2 * H], FP, tag="dhc")  # th|hn
            nc.scalar.activation(dhc[:, 0:H], dcn[:], AF.Tanh)
            nc.vector.tensor_mul(dhc[:, H:2 * H], dact[:, 3 * H:4 * H],
                                 dhc[:, 0:H])
            dsp = pt.tile([128, 8], FP, tag="tp")
            for j in range(4):
                nc.tensor.matmul(dsp[:, j:j + 1],
                                 dcn[:, 128 * j:128 * (j + 1)], id1,
                                 is_transpose=True, start=True, stop=True)
                nc.tensor.matmul(dsp[:, 4 + j:5 + j],
                                 dhc[:, H + 128 * j:H + 128 * (j + 1)], id1,
                                 is_transpose=True, start=True, stop=True)
            dstaten = spool.tile([128, 8], BF, tag="dstate")
            nc.vector.tensor_copy(dstaten[:], dsp[:])
            yp = pd.tile([1, D], FP, tag="dg")
            for k in range(4):
                nc.tensor.matmul(yp[:], dstaten[:, 4 + k:5 + k],
                                 s_wdoT[:, D * k:D * (k + 1)],
                                 start=(k == 0), stop=False)
            nc.tensor.matmul(yp[:], s_onesb[0:1, 0:1], s_wdob[:],
                             start=False, stop=True)
            nc.scalar.activation(youts[:, D * t:D * (t + 1)], yp[:], AF.Copy)

            h0, c0, h1, c1, h0T, h1T = h0n, c0n, h1n, c1n, h0Tn, h1Tn
            dc, dstate = dcn, dstaten

        if dec_split and not no_dec:
            wlast = nc.tensor.wait_ge(rsem, 2 * n_steps)
            drl = drecvA if n_steps % 2 == 0 else drecvB
            ykt = [dmine[:, 2:3], dmine[:, 3:4], drl[:, 2:3], drl[:, 3:4]]
            yp = pd.tile([1, D], FP, tag="dg")
            for k in range(4):
                mm = nc.tensor.matmul(yp[:], ykt[k],
                                      s_wdoT[:, D * k:D * (k + 1)],
                                      start=(k == 0), stop=False)
                if k >= 2:
                    tile.add_dep_helper(mm.ins, wlast.ins, sync=True,
                                        reason="y tail drecv gated")
            nc.tensor.matmul(yp[:], s_onesb[0:1, 0:1], s_wdob[:],
                             start=False, stop=True)
            nc.scalar.activation(
                youts[:, D * (n_steps - 1):D * n_steps], yp[:], AF.Copy)
        if not no_dec:
            nc.sync.dma_start(out=o_rec[:], in_=youts[:])

        # ---------------- end phase ----------------
        rs_in = dram.tile([BT, H], FP)
        rs_out = dram.tile([64, H], FP)
        nc.sync.dma_start(out=rs_in[:], in_=h1[:])
        if sim_mode:
            nc.sync.dma_start(out=rs_out[:], in_=rs_in[0:64, :])
        else:
            nc.gpsimd.collective_compute(
                "ReduceScatter", ALU.add,
                replica_groups=[[0, 4], [1, 5], [2, 6], [3, 7]],
                ins=[rs_in.opt()], outs=[rs_out.opt()])
        h2x = small.tile([64, H], FP, tag="h2x")
        nc.sync.dma_start(out=h2x[:], in_=rs_out[:])
        hh = small.tile([64, H], FP, tag="hh")
        nc.vector.tensor_scalar_mul(hh[:], h2x[:], 0.5)
        nc.sync.dma_start(out=o_imp[:], in_=hh[:])

        hT = small.tile([128, 4 * 64], FP, tag="hT")
        for k in range(4):
            p = pt.tile([128, 64], FP, tag="tp")
            nc.tensor.matmul(p[:], hh[:, 128 * k:128 * (k + 1)],
                             s_ident[0:64, 0:64],
                             is_transpose=True, start=True, stop=True)
            nc.scalar.activation(hT[:, 64 * k:64 * (k + 1)], p[:], AF.Copy)

        s_fcWT = load(fcWT)
        s_fcb = load(fcb)
        s_onesf = load(ones_f)
        hTB = small.tile([128, 4 * 64], BF, tag="hTB")
        nc.vector.tensor_copy(hTB[:], hT[:])
        lp = pg.tile([64, 512], FP, tag="g")
        for k in range(4):
            nc.tensor.matmul(lp[:], hTB[:, 64 * k:64 * (k + 1)],
                             s_fcWT[:, H * k:H * (k + 1)],
                             start=(k == 0), stop=False)
        nc.tensor.matmul(lp[:], s_onesf[:, 0:64], s_fcb[:],
                         start=False, stop=True)
        lat = small.tile([64, H], FP, tag="lat")
        nc.scalar.activation(lat[:], lp[:], AF.Copy)
        nc.sync.dma_start(out=o_lat[:], in_=lat[:])

        # imputed.T = hT + mT*(vT - hT)
        s_vT = load(vT)
        s_mT = load(mT)
        impT = small.tile([128, 4 * 64], FP, tag="impT")
        nc.vector.tensor_sub(impT[:], s_vT[:], hT[:])
        nc.vector.tensor_mul(impT[:], s_mT[:], impT[:])
        nc.vector.tensor_add(impT[:], hT[:], impT[:])

        # ---- disc chain: s-groups packed at partition bases 0/32/64 ----
        s_w0 = [load(w) for w in dw0]
        s_dwg = [[load(w) for w in row] for row in dwg]
        s_dwo3 = load(dwo3)
        s_dbs = [load(bb) for bb in dbs]
        s_dbo = load(dbo)
        impB = small.tile([128, 4 * 64], BF, tag="impB")
        nc.vector.tensor_copy(impB[:], impT[:])
        # xT[d, 64s+b] = imputed[b, 8s+d], built by strided DMAs
        xT = small.tile([D, 64 * S], BF, tag="xT")
        for d in range(D):
            for kk in range(4):
                nc.sync.dma_start(
                    out=xT[d:d + 1, 1024 * kk:1024 * (kk + 1)],
                    in_=impB[d:128:8, 64 * kk:64 * (kk + 1)])
        pools = {0: pg, 1: pt, 2: pd}
        for batch in ((0, 1, 2), (3, 4, 5), (6, 7)):
            hprev = None
            for ci in range(5):
                hsz = DISC_SZ[ci][1]
                insz = DISC_SZ[ci][0]
                tags = {0: "g", 1: "tp", 2: "dg"}
                gt = [pools[gx].tile([96, 512], FP, tag=tags[gx],
                                     name=f"c{ci}g{gx}") for gx in range(3)]
                for bi, grp in enumerate(batch):
                    for gx in range(3):
                        if ci == 0:
                            nc.tensor.matmul(
                                gt[gx][32 * bi:32 * bi + 32, :],
                                s_w0[gx][:],
                                xT[:, 512 * grp:512 * (grp + 1)],
                                start=True, stop=True,
                                tile_position=(0, 32 * bi))
                        else:
                            sl = slice(32 * bi, 32 * bi + insz)
                            nc.tensor.matmul(
                                gt[gx][32 * bi:32 * bi + hsz, :],
                                s_dwg[ci - 1][gx][sl, :],
                                hprev[sl, :],
                                start=True, stop=True,
                                tile_position=(32 * bi, 32 * bi))
                si = small.tile([96, 512], BF, tag="dsi")
                so = small.tile([96, 512], BF, tag="dso")
                tgd = small.tile([96, 512], BF, tag="dtg")
                nc.scalar.activation(si[:], gt[0][:], AF.Sigmoid,
                                     bias=s_dbs[ci][:, 0:1])
                nc.scalar.activation(so[:], gt[1][:], AF.Sigmoid,
                                     bias=s_dbs[ci][:, 1:2])
                nc.scalar.activation(tgd[:], gt[2][:], AF.Tanh,
                                     bias=s_dbs[ci][:, 2:3])
                c2 = small.tile([96, 512], BF, tag="dc2")
                nc.vector.tensor_mul(c2[:], si[:], tgd[:])
                nc.scalar.activation(c2[:], c2[:], AF.Tanh)
                hnew = small.tile([96, 512], BF, tag="dh")
                nc.vector.tensor_mul(hnew[:], so[:], c2[:])
                hprev = hnew
            for bi, grp in enumerate(batch):
                sl = slice(32 * bi, 32 * bi + 32)
                po = pg.tile([D, 512], FP, tag="g")
                nc.tensor.matmul(po[:], s_dwo3[sl, :], hprev[sl, :],
                                 start=True, stop=True,
                                 tile_position=(32 * bi, 0))
                dout = small.tile([D, 512], FP, tag="dout")
                nc.scalar.activation(dout[:], po[:], AF.Identity,
                                     bias=s_dbo[:])
                nc.sync.dma_start(out=o_disc[:, 512 * grp:512 * (grp + 1)],
                                  in_=dout[:])

    nc.compile()
    return nc


# ======================= host side =======================

def _np(x):
    return np.asarray(x, dtype=np.float32)


def _reorder_gates(m, order=(1, 0, 2, 3)):
    """[i,f,g,o] blocks -> [f,i,g,o] along axis 0."""
    blocks = np.split(m, 4, axis=0)
    return np.concatenate([blocks[j] for j in order], axis=0)


def _sig(z):
    return 1.0 / (1.0 + np.exp(-np.clip(z, -60, 60)))


def _bf(x):
    return np.asarray(x, dtype=NPB)


def _ktiles(wT, n_k, width):
    """wT [n_k*128, width] -> [128, n_k*width] (tile k at cols k*width)."""
    out = np.empty((128, n_k * width), wT.dtype)
    for k in range(n_k):
        out[:, k * width:(k + 1) * width] = wT[128 * k:128 * (k + 1)]
    return out


def _lstm_host(p, x, h, c):
    g = x @ _np(p["Wih"]).T + _np(p["bih"]) + h @ _np(p["Whh"]).T + _np(p["bhh"])
    i, f, gg, o = np.split(g, 4, axis=-1)
    c2 = _sig(f) * c + _sig(i) * np.tanh(gg)
    return _sig(o) * np.tanh(c2), c2


def _prep_core_inputs(values, masks_f, params):
    flip = np.arange(H - 1, -1, -1)

    def gen_dir_weights(gp, permute_l1):
        Wih0 = _reorder_gates(_np(gp[0]["Wih"]))      # [2048, 8]
        b0 = _reorder_gates(_np(gp[0]["bih"]) + _np(gp[0]["bhh"]))
        Wih1 = _np(gp[1]["Wih"]).copy()               # [2048, 512] (in = h0)
        Whh1 = _np(gp[1]["Whh"]).copy()
        b1 = _np(gp[1]["bih"]) + _np(gp[1]["bhh"])
        if permute_l1:
            for bidx in range(4):
                sl = slice(bidx * H, (bidx + 1) * H)
                Wih1[sl] = Wih1[sl][flip]
                Whh1[sl] = Whh1[sl][flip]
                b1[sl] = b1[sl][flip]
            Whh1 = Whh1[:, flip]
        Wih1 = _reorder_gates(Wih1)
        Whh1 = _reorder_gates(Whh1)
        b1 = _reorder_gates(b1)
        Whh0 = _reorder_gates(_np(gp[0]["Whh"]))
        wih0b = np.zeros((40, GATES), np.float32)
        wih0b[0] = b0
        wih0b[32:40] = Wih0.T
        return (_bf(wih0b), _bf(_ktiles(Whh0.T, 4, GATES)),
                _bf(_ktiles(Wih1.T, 4, GATES)), _bf(_ktiles(Whh1.T, 4, GATES)),
                _bf(b1[None, :]))

    def warmup(gp, permute):
        h1v, c1v = _lstm_host(gp[1], np.zeros((1, H), np.float32),
                              np.zeros((1, H), np.float32),
                              np.zeros((1, H), np.float32))
        h1v, c1v = h1v[0], c1v[0]
        if permute:
            h1v, c1v = h1v[flip], c1v[flip]
        return h1v, c1v

    def go_weights(permute):
        W = _np(params["gen_out"]["W"]).copy()        # [8, 512]
        if permute:
            W = W[:, flip]
        return (_bf(_ktiles(np.ascontiguousarray(W.T), 4, D)),
                _bf(_np(params["gen_out"]["b"])[None, :]))

    # decoder (shared)
    dp = params["dec_cell"]
    start = np.full((1, H), 128.0, np.float32)
    hd1, cd1 = _lstm_host(dp, start, np.zeros((1, H), np.float32),
                          np.zeros((1, H), np.float32))
    Wcat = np.concatenate([_np(dp["Wih"]), _np(dp["Whh"])], axis=1)
    Wcat = _reorder_gates(Wcat)           # rows [f,i,g,o] x 512; cols [c|h]
    db = _reorder_gates(_np(dp["bih"]) + _np(dp["bhh"]))
    wdob = _bf(_np(params["dec_out"]["b"])[None, :])
    Wdo = _np(params["dec_out"]["W"])     # [8, 512]

    def dec_half(lo):
        # my gate rows: [f,i,g,o] restricted to hid [lo, lo+256)
        rows = np.concatenate([np.arange(g * H + lo, g * H + lo + 256)
                               for g in range(4)])
        Wm = Wcat[rows]                   # [1024, 1024]
        bm = db[rows]
        # state col order: c_my0,c_my1,c_par0,c_par1,h_my0,h_my1,h_par0,h_par1
        po = 256 - lo
        cols = np.concatenate([
            np.arange(lo, lo + 256), np.arange(po, po + 256),
            np.arange(512 + lo, 512 + lo + 256),
            np.arange(512 + po, 512 + po + 256)])
        WmT = np.ascontiguousarray(Wm[:, cols].T)    # [1024, 1024]
        wdecT_ = _bf(_ktiles(WmT, 8, GATES // 2))
        wdec_b_ = _bf(bm[None, :])
        dmine_ = np.concatenate(
            [cd1[0, lo:lo + 256].reshape(2, 128).T,
             hd1[0, lo:lo + 256].reshape(2, 128).T], axis=1)
        drecv_ = np.concatenate(
            [cd1[0, po:po + 256].reshape(2, 128).T,
             hd1[0, po:po + 256].reshape(2, 128).T], axis=1)
        wdoT_ = _bf(_ktiles(np.ascontiguousarray(
            np.concatenate([Wdo[:, lo:lo + 256], Wdo[:, po:po + 256]],
                           axis=1).T), 4, D))
        return (wdecT_, wdec_b_, _bf(dmine_), _bf(drecv_),
                cd1[0:1, lo:lo + 256].astype(np.float32), wdoT_)

    dec_lo = dec_half(0)
    dec_hi = dec_half(256)
    # solo-decoder (dec_split=False) arrays
    wdecT_solo = _bf(_ktiles(np.ascontiguousarray(Wcat.T), 8, GATES))
    wdecb_solo = _bf(db[None, :])
    dstate0_solo = _bf(np.concatenate(
        [cd1.reshape(4, 128).T, hd1.reshape(4, 128).T], axis=1))
    decc1_solo = cd1.reshape(1, H).astype(np.float32)
    wdoT_solo = _bf(_ktiles(np.ascontiguousarray(Wdo.T), 4, D))

    # disc (shared): pruned cells; gate blocks i/o/g
    def disc_w(i):
        p = params["disc"][i]
        W = _np(p["Wih"])
        b = _np(p["bih"]) + _np(p["bhh"])
        hsz = DISC_SZ[i][1]
        return {n: (W[bi * hsz:(bi + 1) * hsz], b[bi * hsz:(bi + 1) * hsz])
                for n, bi in (("i", 0), ("g", 2), ("o", 3))}

    b1_ = disc_w(0)
    dw0_in = [_bf(np.ascontiguousarray(b1_[name][0].T))
              for name in ("i", "o", "g")]
    dwg_in, dbs_in = [], []
    db0 = np.zeros((96, 3), np.float32)
    for bpos in range(3):
        for gx, name in enumerate(("i", "o", "g")):
            db0[32 * bpos:32 * bpos + 32, gx] = b1_[name][1]
    dbs_in.append(db0)
    for i in range(1, 5):
        blocks = disc_w(i)
        insz, hsz = DISC_SZ[i]
        row = []
        dbi = np.zeros((96, 3), np.float32)
        for gx, name in enumerate(("i", "o", "g")):
            w = np.zeros((96, hsz), np.float32)
            for bpos in range(3):
                w[32 * bpos:32 * bpos + insz] = blocks[name][0].T
                dbi[32 * bpos:32 * bpos + hsz, gx] = blocks[name][1]
            row.append(_bf(w))
        dwg_in.append(row)
        dbs_in.append(dbi)
    dwo3 = np.zeros((96, D), np.float32)
    for bpos in range(3):
        dwo3[32 * bpos:32 * bpos + 32] = _np(params["disc_out"]["W"]).T
    dwo3 = _bf(dwo3)
    dbo = _np(params["disc_out"]["b"])[:, None]

    fcWT = _bf(_ktiles(np.ascontiguousarray(_np(params["fc"]["W"]).T), 4, H))
    fcb = _bf(_np(params["fc"]["b"])[None, :])

    shared = dict(
        ident=np.eye(128, dtype=np.float32),
        ones_bf=_bf(np.ones((1, 128), np.float32)),
        ones_f=_bf(np.ones((1, 128), np.float32)),
        wdob=wdob,
        fcWT=fcWT, fcb=fcb,
        dwo3=dwo3, dbo=dbo,
        **{f"dw0_{g}": dw0_in[g] for g in range(3)},
        **{f"dw{i}_{g}": dwg_in[i - 1][g] for i in range(1, 5)
           for g in range(3)},
        **{f"db{i}": dbs_in[i] for i in range(5)},
    )

    fwd_w = gen_dir_weights(params["gen_fwd"], False)
    bwd_w = gen_dir_weights(params["gen_bwd"], True)
    fwd_h1, fwd_c1 = warmup(params["gen_fwd"], False)
    bwd_h1, bwd_c1 = warmup(params["gen_bwd"], True)
    fwd_go = go_weights(False)
    bwd_go = go_weights(True)

    in_maps = []
    for core in range(NCORES):
        is_bwd = core >= 4
        ct = core % 4
        rows = slice(128 * ct, 128 * (ct + 1))
        v = values[rows]
        m = masks_f[rows]
        if is_bwd:
            v = v[:, ::-1]
            m = m[:, ::-1]
        xq_ = (m * v).reshape(BT, S * D).astype(np.float32)
        xom_ = (1.0 - m).reshape(BT, S * D).astype(np.float32)
        (wih0b, whh0T, wih1T, whh1T, b1row) = bwd_w if is_bwd else fwd_w
        h1v, c1v = (bwd_h1, bwd_c1) if is_bwd else (fwd_h1, fwd_c1)
        goT_, gob_ = bwd_go if is_bwd else fwd_go
        h1iT = np.repeat(h1v.reshape(4, 128).T[:, :, None], BT, 2)
        h1iT = h1iT.reshape(128, 4 * BT)
        r0 = 128 * ct + (64 if is_bwd else 0)
        ve = values[r0:r0 + 64].reshape(64, 512)
        me = masks_f[r0:r0 + 64].reshape(64, 512)
        vT_ = _ktiles(np.ascontiguousarray(ve.T), 4, 64)
        mT_ = _ktiles(np.ascontiguousarray(me.T), 4, 64)
        in_maps.append(dict(
            wdecT=wdecT_solo, wdec_b=wdecb_solo, dstate0=dstate0_solo,
            dec_c1=decc1_solo, wdoT=wdoT_solo,
            xq=np.ascontiguousarray(xq_), xom=np.ascontiguousarray(xom_),
            wih0b=wih0b, whh0T=whh0T, wih1T=wih1T, whh1T=whh1T, b1row=b1row,
            goT=goT_, gob=gob_,
            h1i=np.tile(h1v, (BT, 1)).astype(np.float32),
            c1i=np.tile(c1v, (BT, 1)).astype(np.float32),
            h1iT=_bf(h1iT),
            vT=np.ascontiguousarray(vT_), mT=np.ascontiguousarray(mT_),
            **shared))
    return in_maps


_CACHE = {}


def kernel(values, masks, params):
    values = _np(values)
    masks_i = np.asarray(masks)
    masks_f = masks_i.astype(np.float32)

    if "nc" not in _CACHE:
        _CACHE["nc"] = build_program()
    nc = _CACHE["nc"]

    in_maps = _prep_core_inputs(values, masks_f, params)
    from concourse.bass_utils import run_bass_kernel_spmd
    res = run_bass_kernel_spmd(nc, in_maps, list(range(NCORES)))
    results = res.results

    impute = np.empty((B, H), np.float32)
    latent = np.empty((B, H), np.float32)
    disc = np.empty((B, S, D), np.float32)
    for core in range(NCORES):
        ct = core % 4
        r0 = 128 * ct + (64 if core >= 4 else 0)
        impute[r0:r0 + 64] = results[core]["o_imp"]
        latent[r0:r0 + 64] = results[core]["o_lat"]
        od = results[core]["o_disc"]          # [8, 4096] cols = 64*s + b
        disc[r0:r0 + 64] = od.reshape(D, S, 64).transpose(2, 1, 0)
    rec_row = results[0]["o_rec"].reshape(S, D)
    reconstructed = np.broadcast_to(rec_row, (B, S, D)).copy()
    impute = impute.reshape(B, S, D)
    return disc, impute, latent, reconstructed


# revision 26
# speedup vs baseline: 1.0277x; 1.0277x over previous
"""CRLI kernel for trn2, 8 NeuronCores (SPMD, one chip).

Sharding: 8 independent generator scans = (fwd/bwd) x (4 batch tiles of 128).
Cores 0-3: forward direction on batch rows [128c, 128c+128); cores 4-7:
backward direction (host time-reverses inputs) on the same rows.  The program
is identical on all cores; direction / batch tile comes from per-core inputs.

All generator/decoder matmuls are activation-stationary: lhsT = transposed
activations [K=feature, M=batch<=128] in bf16, rhs = transposed weights in
bf16 streaming at N=512.  PSUM accumulates fp32; the c-state path stays fp32
(measured end-to-end bf16 error ~1e-3 << 2e-2 tolerance).

The batch-constant decoder (64 autonomous steps) is replicated on every core
and interleaved into the scan.  The final h = (hf + flip(hb))/2 combine is a
pairwise ReduceScatter; the hidden flip is pre-applied on the host by
permuting the bwd layer-1 weights.  The discriminator (zero-state LSTM cells
== feedforward, f-gate pruned) runs 8-way batch-split; its i/o/g gate chains
are kept on identical partition lanes, with 3 independent s-groups packed at
partition bases 0/32/64 via tile_position col-tiling.
"""

import numpy as np
import ml_dtypes

import concourse.bass as bass
import concourse.bacc as bacc
import concourse.mybir as mybir
import concourse.tile as tile

FP = mybir.dt.float32
FR = mybir.dt.float32r
BF = mybir.dt.bfloat16
AF = mybir.ActivationFunctionType
ALU = mybir.AluOpType

B, S, D, H = 512, 64, 8, 512
GATES = 4 * H          # 2048
NB = GATES // 512      # psum banks per gate set
BT = 128               # batch rows per core
NCORES = 8
NPB = ml_dtypes.bfloat16

# disc: (in, hid) per pruned cell
DISC_SZ = [(8, 32), (32, 16), (16, 8), (8, 16), (16, 32)]


def fr(ap):
    return ap.bitcast(FR)


class _PrechargeRemoteSem:
    """Tile's schedule-time CoreSim is single-core: the partner's
    remote_dma sem increments never arrive, so waits on dec_rsem would
    deadlock the scheduling pass.  Pre-charge that sem in the scheduling
    sim only -- the emitted program is unchanged (sems are cleared by the
    kernel preamble at runtime)."""

    def __init__(self, sem_holder):
        self.sem_holder = sem_holder

    def __enter__(self):
        import concourse.bass_interp as BI
        from concourse.bass import create_sync_update
        self.BI = BI
        self.orig = BI.CoreSim.simulate
        holder = self.sem_holder

        def patched(sim_self, *a, **k):
            sem = holder.get("rsem")
            if sem is not None:
                try:
                    for _ in range(2 * S + 4):
                        sim_self.update_semaphore(create_sync_update(sem, 2))
                except Exception:
                    pass
            return self.orig(sim_self, *a, **k)

        BI.CoreSim.simulate = patched
        return self

    def __exit__(self, *a):
        self.BI.CoreSim.simulate = self.orig


def build_program(n_steps=S, sim_mode=False, no_dec=False, no_disc=False,
                  dec_split=False):
    holder = {}
    with _PrechargeRemoteSem(holder):
        return _build_program(n_steps, sim_mode, no_dec, no_disc, dec_split,
                              holder)


def _build_program(n_steps, sim_mode, no_dec, no_disc, dec_split, holder):
    nc = bacc.Bacc("TRN2", target_bir_lowering=False, debug=False)

    def din(name, shape, d=FP):
        return nc.dram_tensor(name, shape, d, kind="ExternalInput")

    xq = din("xq", [BT, S * D])            # mask * values (direction-ordered)
    xom = din("xom", [BT, S * D])          # 1 - mask
    wih0b = din("wih0b", [40, GATES], BF)  # row0 bias, rows 32-39 Wih0.T
    whh0T = din("whh0T", [128, 4 * GATES], BF)
    wih1T = din("wih1T", [128, 4 * GATES], BF)
    whh1T = din("whh1T", [128, 4 * GATES], BF)
    b1row = din("b1row", [1, GATES], BF)
    goT = din("goT", [128, 4 * D], BF)
    gob = din("gob", [1, D], BF)
    h1i = din("h1i", [BT, H])
    c1i = din("c1i", [BT, H])
    h1iT = din("h1iT", [128, 4 * BT], BF)
    ones_bf = din("ones_bf", [1, 128], BF)
    ident = din("ident", [128, 128])
    # decoder
    if dec_split:
        wdecT = din("wdecT", [128, 8 * GATES // 2], BF)
        wdec_b = din("wdec_b", [1, GATES // 2], BF)
        dmine0 = din("dmine0", [128, 4], BF)   # [c_my0,c_my1,h_my0,h_my1]
        drecv0 = din("drecv0", [128, 4], BF)   # partner halves
        dec_c1 = din("dec_c1", [1, H // 2])
    else:
        wdecT = din("wdecT", [128, 8 * GATES], BF)
        wdec_b = din("wdec_b", [1, GATES], BF)
        dstate0 = din("dstate0", [128, 8], BF)
        dec_c1 = din("dec_c1", [1, H])
    wdoT = din("wdoT", [128, 4 * D], BF)
    wdob = din("wdob", [1, D], BF)
    # end phase
    vT = din("vT", [128, 4 * 64])
    mT = din("mT", [128, 4 * 64])
    fcWT = din("fcWT", [128, 4 * H], BF)
    fcb = din("fcb", [1, H], BF)
    ones_f = din("ones_f", [1, 128], BF)
    dw0 = [din(f"dw0_{g}", [D, 32], BF) for g in range(3)]
    dwg = [[din(f"dw{i}_{g}", [96, DISC_SZ[i][1]], BF) for g in range(3)]
           for i in range(1, 5)]
    dwo3 = din("dwo3", [96, D], BF)
    dbs = [din(f"db{i}", [96, 3]) for i in range(5)]
    dbo = din("dbo", [D, 1])

    o_imp = nc.dram_tensor("o_imp", [64, H], FP, kind="ExternalOutput")
    o_disc = nc.dram_tensor("o_disc", [D, 64 * S], FP, kind="ExternalOutput")
    o_lat = nc.dram_tensor("o_lat", [64, H], FP, kind="ExternalOutput")
    o_rec = nc.dram_tensor("o_rec", [1, S * D], FP, kind="ExternalOutput")

    with tile.TileContext(nc) as tc, \
         tc.tile_pool(name="wpool", bufs=1) as wpool, \
         tc.tile_pool(name="state", bufs=2) as spool, \
         tc.tile_pool(name="small", bufs=1) as small, \
         tc.tile_pool(name="gates", bufs=4, space="PSUM") as pg, \
         tc.tile_pool(name="tpose", bufs=2, space="PSUM") as pt, \
         tc.tile_pool(name="dgates", bufs=2, space="PSUM") as pd, \
         tc.tile_pool(name="dram", bufs=1, space="DRAM") as dram:

        def load(t, pool=wpool):
            st = pool.tile(list(t.shape), t.dtype, tag=f"w_{t.name}",
                           name=f"s_{t.name}")
            nc.sync.dma_start(out=st[:], in_=t[:])
            return st

        s_xq = load(xq)
        s_xom = load(xom)
        s_wih0b = load(wih0b)
        s_whh0T = load(whh0T)
        s_goT = load(goT)
        s_gob = load(gob)
        s_onesb = load(ones_bf)
        s_ident = load(ident)
        s_wih1T = load(wih1T)
        s_whh1T = load(whh1T)
        s_b1row = load(b1row)
        s_wdecT = load(wdecT)
        s_wdec_b = load(wdec_b)
        s_wdoT = load(wdoT)
        s_wdob = load(wdob)
        id1 = s_ident[0:1, 0:1]

        h0 = spool.tile([BT, H], FP, tag="h0")
        c0 = spool.tile([BT, H], FP, tag="c0")
        h1 = spool.tile([BT, H], FP, tag="h1")
        c1 = spool.tile([BT, H], FP, tag="c1")
        h0T = spool.tile([128, 4 * BT], BF, tag="h0T")
        h1T = spool.tile([128, 4 * BT], BF, tag="h1T")
        nc.vector.memset(h0[:], 0.0)
        nc.vector.memset(c0[:], 0.0)
        nc.vector.memset(h0T[:], 0.0)
        nc.sync.dma_start(out=h1[:], in_=h1i[:])
        nc.sync.dma_start(out=c1[:], in_=c1i[:])
        nc.sync.dma_start(out=h1T[:], in_=h1iT[:])

        if dec_split:
            dmine = spool.tile([128, 4], BF, tag="dmine")
            nc.sync.dma_start(out=dmine[:], in_=dmine0[:])
            drecvA = wpool.tile([128, 4], BF)
            drecvB = wpool.tile([128, 4], BF)
            nc.sync.dma_start(out=drecvA[:], in_=drecv0[:])
            dc = spool.tile([1, H // 2], FP, tag="dc")
            nc.sync.dma_start(out=dc[:], in_=dec_c1[:])
            rsem = nc.alloc_semaphore("dec_rsem")
            lsem = nc.alloc_semaphore("dec_lsem")
            if not sim_mode:
                holder["rsem"] = rsem
        else:
            dstate = spool.tile([128, 8], BF, tag="dstate")
            nc.sync.dma_start(out=dstate[:], in_=dstate0[:])
            dc = spool.tile([1, H], FP, tag="dc")
            nc.sync.dma_start(out=dc[:], in_=dec_c1[:])
        youts = wpool.tile([1, S * D], FP)

        def cell_acts(gb, c_prev, h_new, c_new):
            """gb: 4 psum [BT,512] banks in order f,i,g,o."""
            sf = small.tile([BT, 512], FP, tag="sf")
            si = small.tile([BT, 512], FP, tag="si")
            tg = small.tile([BT, 512], FP, tag="tg")
            so = small.tile([BT, 512], FP, tag="so")
            nc.scalar.activation(sf[:], gb[0][:], AF.Sigmoid)
            nc.scalar.activation(si[:], gb[1][:], AF.Sigmoid)
            nc.scalar.activation(tg[:], gb[2][:], AF.Tanh)
            nc.scalar.activation(so[:], gb[3][:], AF.Sigmoid)
            t1 = small.tile([BT, 512], FP, tag="t1")
            t2 = small.tile([BT, 512], FP, tag="t2")
            nc.vector.tensor_mul(t1[:], sf[:], c_prev[:])
            nc.vector.tensor_mul(t2[:], si[:], tg[:])
            nc.vector.tensor_add(c_new[:], t1[:], t2[:])
            th = small.tile([BT, 512], FP, tag="th")
            nc.scalar.activation(th[:], c_new[:], AF.Tanh)
            nc.vector.tensor_mul(h_new[:], so[:], th[:])

        def transpose_to_bf(src, dstT):
            """src [BT,512] fp32 -> dstT [128, 4*BT] bf16 (4 PE transposes)."""
            for k in range(4):
                p = pt.tile([128, BT], FP, tag="tp", name=f"tp{k}")
                nc.tensor.matmul(p[:], src[:, 128 * k:128 * (k + 1)],
                                 s_ident[:], is_transpose=True,
                                 start=True, stop=True)
                nc.scalar.activation(dstT[:, BT * k:BT * (k + 1)], p[:],
                                     AF.Copy)

        for t in range(n_steps):
            # ---- ximp = h1 @ gen_out.W.T + b ----
            xp = pt.tile([BT, D], FP, tag="tp")
            for k in range(4):
                nc.tensor.matmul(xp[:], h1T[:, BT * k:BT * (k + 1)],
                                 s_goT[:, D * k:D * (k + 1)],
                                 start=(k == 0), stop=False)
            nc.tensor.matmul(xp[:], s_onesb[:, 0:BT], s_gob[:],
                             start=False, stop=True)
            # in_f = (1-m)*ximp + m*x
            inf = small.tile([BT, D], FP, tag="inf")
            nc.vector.tensor_mul(inf[:], s_xom[:, D * t:D * (t + 1)], xp[:])
            nc.vector.tensor_add(inf[:], inf[:], s_xq[:, D * t:D * (t + 1)])
            infT = small.tile([40, 128], BF, tag="infT")
            pti = pt.tile([D, 128], FP, tag="tp")
            nc.tensor.matmul(pti[:], inf[:], s_ident[:], is_transpose=True,
                             start=True, stop=True)
            nc.vector.memset(infT[0:32, :], 1.0)
            nc.scalar.activation(infT[32:32 + D, :], pti[:], AF.Copy)

            # ---- layer 0 gates (f,i,g,o banks) ----
            g0b = []
            for bk in range(NB):
                gb = pg.tile([BT, 512], FP, tag="g", name=f"g0_{bk}")
                for k in range(4):
                    nc.tensor.matmul(gb[:], h0T[:, BT * k:BT * (k + 1)],
                                     s_whh0T[:, GATES * k + 512 * bk:
                                             GATES * k + 512 * (bk + 1)],
                                     start=(k == 0), stop=False)
                nc.tensor.matmul(gb[:], infT[:],
                                 s_wih0b[:, 512 * bk:512 * (bk + 1)],
                                 start=False, stop=True)
                g0b.append(gb)
            h0n = spool.tile([BT, H], FP, tag="h0")
            c0n = spool.tile([BT, H], FP, tag="c0")
            cell_acts(g0b, c0, h0n, c0n)
            h0Tn = spool.tile([128, 4 * BT], BF, tag="h0T")
            transpose_to_bf(h0n, h0Tn)

            # ---- layer 1 gates ----
            g1b = []
            for bk in range(NB):
                gb = pg.tile([BT, 512], FP, tag="g", name=f"g1_{bk}")
                for k in range(4):
                    nc.tensor.matmul(gb[:], h1T[:, BT * k:BT * (k + 1)],
                                     s_whh1T[:, GATES * k + 512 * bk:
                                             GATES * k + 512 * (bk + 1)],
                                     start=(k == 0), stop=False)
                for k in range(4):
                    nc.tensor.matmul(gb[:], h0Tn[:, BT * k:BT * (k + 1)],
                                     s_wih1T[:, GATES * k + 512 * bk:
                                             GATES * k + 512 * (bk + 1)],
                                     start=False, stop=False)
                last_g1_mm = nc.tensor.matmul(
                    gb[:], s_onesb[:, 0:128],
                    s_b1row[:, 512 * bk:512 * (bk + 1)],
                    start=False, stop=True)
                g1b.append(gb)
            h1n = spool.tile([BT, H], FP, tag="h1")
            c1n = spool.tile([BT, H], FP, tag="c1")
            cell_acts(g1b, c1, h1n, c1n)
            h1Tn = spool.tile([128, 4 * BT], BF, tag="h1T")
            transpose_to_bf(h1n, h1Tn)

            if no_dec:
                h0, c0, h1, c1, h0T, h1T = h0n, c0n, h1n, c1n, h0Tn, h1Tn
                continue
            if dec_split:
                # ---- decoder step, pair-split (my 1024 gates) ----
                HQ = H // 2
                drd = drecvA if t % 2 == 0 else drecvB
                dwr = drecvB if t % 2 == 0 else drecvA
                # state K-tile order: c_my0,c_my1,c_par0,c_par1,h_my0,...
                kt = [dmine[:, 0:1], dmine[:, 1:2], drd[:, 0:1], drd[:, 1:2],
                      dmine[:, 2:3], dmine[:, 3:4], drd[:, 2:3], drd[:, 3:4]]
                last_gen = None
                dg = []
                drecv_mms = []
                for bk in range(2):
                    gb = pd.tile([1, 512], FP, tag="dg", name=f"dg{bk}")
                    for k in range(8):
                        mm = nc.tensor.matmul(
                            gb[:], kt[k],
                            s_wdecT[:, 1024 * k + 512 * bk:
                                    1024 * k + 512 * (bk + 1)],
                            start=(k == 0), stop=False)
                        if k in (2, 3, 6, 7):
                            drecv_mms.append(mm)
                    nc.tensor.matmul(gb[:], s_onesb[0:1, 0:1],
                                     s_wdec_b[:, 512 * bk:512 * (bk + 1)],
                                     start=False, stop=True)
                    dg.append(gb)
                if t > 0:
                    w = nc.tensor.wait_ge(rsem, 2 * t)
                    tile.add_dep_helper(w.ins, last_g1_mm.ins, sync=True,
                                        reason="dec wait after gen mms")
                    for mm in drecv_mms:
                        tile.add_dep_helper(mm.ins, w.ins, sync=True,
                                            reason="dec drecv gated")
                # acts: bank0 = [f|i], bank1 = [g|o]
                dact = small.tile([1, 4 * HQ], FP, tag="dact")
                nc.scalar.activation(dact[:, 0:2 * HQ], dg[0][:], AF.Sigmoid)
                nc.scalar.activation(dact[:, 2 * HQ:3 * HQ],
                                     dg[1][:, 0:HQ], AF.Tanh)
                nc.scalar.activation(dact[:, 3 * HQ:4 * HQ],
                                     dg[1][:, HQ:2 * HQ], AF.Sigmoid)
                dmix = small.tile([1, 2 * HQ], FP, tag="dmix")
                nc.vector.tensor_mul(dmix[:, 0:HQ], dact[:, 0:HQ], dc[:])
                nc.vector.tensor_mul(dmix[:, HQ:2 * HQ], dact[:, HQ:2 * HQ],
                                     dact[:, 2 * HQ:3 * HQ])
                dcn = spool.tile([1, H // 2], FP, tag="dc")
                nc.vector.tensor_add(dcn[:], dmix[:, 0:HQ], dmix[:, HQ:2 * HQ])
                dhc = small.tile([1, 2 * HQ], FP, tag="dhc")
                nc.scalar.activation(dhc[:, 0:HQ], dcn[:], AF.Tanh)
                nc.vector.tensor_mul(dhc[:, HQ:2 * HQ],
                                     dact[:, 3 * HQ:4 * HQ], dhc[:, 0:HQ])
                dsp = pt.tile([128, 4], FP, tag="tp")
                for j in range(2):
                    nc.tensor.matmul(dsp[:, j:j + 1],
                                     dcn[:, 128 * j:128 * (j + 1)], id1,
                                     is_transpose=True, start=True, stop=True)
                    nc.tensor.matmul(dsp[:, 2 + j:3 + j],
                                     dhc[:, HQ + 128 * j:HQ + 128 * (j + 1)],
                                     id1, is_transpose=True,
                                     start=True, stop=True)
                dminen = spool.tile([128, 4], BF, tag="dmine")
                nc.vector.tensor_copy(dminen[:], dsp[:])
                if sim_mode:
                    nc.vector.tensor_copy(dwr[:], dminen[:])
                    nc.gpsimd.sem_inc(rsem, 2)
                else:
                    nc.gpsimd.remote_dma_broadcast(
                        dwr[:], dminen[:], rsem, lsem,
                        rdests=[None, None, None, None,
                                (0, 4), None, None, None])
                    nc.gpsimd.trigger_dma(1)
                # y_{t-1} from full h after step t-1: dmine (pre-update) +
                # drd (exchange t-1) -- both gated by w
                if t > 0:
                    ykt = [dmine[:, 2:3], dmine[:, 3:4],
                           drd[:, 2:3], drd[:, 3:4]]
                    yp = pd.tile([1, D], FP, tag="dg")
                    for k in range(4):
                        mm = nc.tensor.matmul(yp[:], ykt[k],
                                              s_wdoT[:, D * k:D * (k + 1)],
                                              start=(k == 0), stop=False)
                        if k >= 2:
                            tile.add_dep_helper(mm.ins, w.ins, sync=True,
                                                reason="y drecv gated")
                    nc.tensor.matmul(yp[:], s_onesb[0:1, 0:1], s_wdob[:],
                                     start=False, stop=True)
                    nc.scalar.activation(
                        youts[:, D * (t - 1):D * t], yp[:], AF.Copy)
                h0, c0, h1, c1, h0T, h1T = h0n, c0n, h1n, c1n, h0Tn, h1Tn
                dc, dmine = dcn, dminen
                continue
            # ---- decoder step (replicated, bf16) ----
            # ---- decoder step (replicated, bf16) ----
            dg = []
            for bk in range(NB):
                gb = pd.tile([1, 512], FP, tag="dg", name=f"dg{bk}")
                for k in range(8):
                    nc.tensor.matmul(gb[:], dstate[:, k:k + 1],
                                     s_wdecT[:, GATES * k + 512 * bk:
                                             GATES * k + 512 * (bk + 1)],
                                     start=(k == 0), stop=False)
                nc.tensor.matmul(gb[:], s_onesb[0:1, 0:1],
                                 s_wdec_b[:, 512 * bk:512 * (bk + 1)],
                                 start=False, stop=True)
                dg.append(gb)
            dact = small.tile([1, 4 * H], FP, tag="dact")  # sf|si|tg|so
            nc.scalar.activation(dact[:, 0:H], dg[0][:], AF.Sigmoid)
            nc.scalar.activation(dact[:, H:2 * H], dg[1][:], AF.Sigmoid)
            nc.scalar.activation(dact[:, 2 * H:3 * H], dg[2][:], AF.Tanh)
            nc.scalar.activation(dact[:, 3 * H:4 * H], dg[3][:], AF.Sigmoid)
            dmix = small.tile([1, 2 * H], FP, tag="dmix")  # t1|t2
            nc.vector.tensor_mul(dmix[:, 0:H], dact[:, 0:H], dc[:])
            nc.vector.tensor_mul(dmix[:, H:2 * H], dact[:, H:2 * H],
                                 dact[:, 2 * H:3 * H])
            dcn = spool.tile([1, H], FP, tag="dc")
            nc.vector.tensor_add(dcn[:], dmix[:, 0:H], dmix[:, H:2 * H])
            dhc = small.tile([1, 2 * H], FP, tag="dhc")  # th|hn
            nc.scalar.activation(dhc[:, 0:H], dcn[:], AF.Tanh)
            nc.vector.tensor_mul(dhc[:, H:2 * H], dact[:, 3 * H:4 * H],
                                 dhc[:, 0:H])
            dsp = pt.tile([128, 8], FP, tag="tp")
            for j in range(4):
                nc.tensor.matmul(dsp[:, j:j + 1],
                                 dcn[:, 128 * j:128 * (j + 1)], id1,
                                 is_transpose=True, start=True, stop=True)
                nc.tensor.matmul(dsp[:, 4 + j:5 + j],
                                 dhc[:, H + 128 * j:H + 128 * (j + 1)], id1,
                                 is_transpose=True, start=True, stop=True)
            dstaten = spool.tile([128, 8], BF, tag="dstate")
            nc.vector.tensor_copy(dstaten[:], dsp[:])
            yp = pd.tile([1, D], FP, tag="dg")
            for k in range(4):
                nc.tensor.matmul(yp[:], dstaten[:, 4 + k:5 + k],
                                 s_wdoT[:, D * k:D * (k + 1)],
                                 start=(k == 0), stop=False)
            nc.tensor.matmul(yp[:], s_onesb[0:1, 0:1], s_wdob[:],
                             start=False, stop=True)
            nc.scalar.activation(youts[:, D * t:D * (t + 1)], yp[:], AF.Copy)

            h0, c0, h1, c1, h0T, h1T = h0n, c0n, h1n, c1n, h0Tn, h1Tn
            dc, dstate = dcn, dstaten

        if dec_split and not no_dec:
            wlast = nc.tensor.wait_ge(rsem, 2 * n_steps)
            drl = drecvA if n_steps % 2 == 0 else drecvB
            ykt = [dmine[:, 2:3], dmine[:, 3:4], drl[:, 2:3], drl[:, 3:4]]
            yp = pd.tile([1, D], FP, tag="dg")
            for k in range(4):
                mm = nc.tensor.matmul(yp[:], ykt[k],
                                      s_wdoT[:, D * k:D * (k + 1)],
                                      start=(k == 0), stop=False)
                if k >= 2:
                    tile.add_dep_helper(mm.ins, wlast.ins, sync=True,
                                        reason="y tail drecv gated")
            nc.tensor.matmul(yp[:], s_onesb[0:1, 0:1], s_wdob[:],
                             start=False, stop=True)
            nc.scalar.activation(
                youts[:, D * (n_steps - 1):D * n_steps], yp[:], AF.Copy)
        if not no_dec:
            nc.sync.dma_start(out=o_rec[:], in_=youts[:])

        # ---------------- end phase ----------------
        rs_in = dram.tile([BT, H], FP)
        rs_out = dram.tile([64, H], FP)
        nc.sync.dma_start(out=rs_in[:], in_=h1[:])
        if sim_mode:
            nc.sync.dma_start(out=rs_out[:], in_=rs_in[0:64, :])
        else:
            nc.gpsimd.collective_compute(
                "ReduceScatter", ALU.add,
                replica_groups=[[0, 4], [1, 5], [2, 6], [3, 7]],
                ins=[rs_in.opt()], outs=[rs_out.opt()])
        h2x = small.tile([64, H], FP, tag="h2x")
        nc.sync.dma_start(out=h2x[:], in_=rs_out[:])
        hh = small.tile([64, H], FP, tag="hh")
        nc.vector.tensor_scalar_mul(hh[:], h2x[:], 0.5)
        nc.sync.dma_start(out=o_imp[:], in_=hh[:])

        hT = small.tile([128, 4 * 64], FP, tag="hT")
        for k in range(4):
            p = pt.tile([128, 64], FP, tag="tp")
            nc.tensor.matmul(p[:], hh[:, 128 * k:128 * (k + 1)],
                             s_ident[0:64, 0:64],
                             is_transpose=True, start=True, stop=True)
            nc.scalar.activation(hT[:, 64 * k:64 * (k + 1)], p[:], AF.Copy)

        s_fcWT = load(fcWT)
        s_fcb = load(fcb)
        s_onesf = load(ones_f)
        hTB = small.tile([128, 4 * 64], BF, tag="hTB")
        nc.vector.tensor_copy(hTB[:], hT[:])
        lp = pg.tile([64, 512], FP, tag="g")
        for k in range(4):
            nc.tensor.matmul(lp[:], hTB[:, 64 * k:64 * (k + 1)],
                             s_fcWT[:, H * k:H * (k + 1)],
                             start=(k == 0), stop=False)
        nc.tensor.matmul(lp[:], s_onesf[:, 0:64], s_fcb[:],
                         start=False, stop=True)
        lat = small.tile([64, H], FP, tag="lat")
        nc.scalar.activation(lat[:], lp[:], AF.Copy)
        nc.sync.dma_start(out=o_lat[:], in_=lat[:])

        # imputed.T = hT + mT*(vT - hT)
        s_vT = load(vT)
        s_mT = load(mT)
        impT = small.tile([128, 4 * 64], FP, tag="impT")
        nc.vector.tensor_sub(impT[:], s_vT[:], hT[:])
        nc.vector.tensor_mul(impT[:], s_mT[:], impT[:])
        nc.vector.tensor_add(impT[:], hT[:], impT[:])

        # ---- disc chain: s-groups packed at partition bases 0/32/64 ----
        s_w0 = [load(w) for w in dw0]
        s_dwg = [[load(w) for w in row] for row in dwg]
        s_dwo3 = load(dwo3)
        s_dbs = [load(bb) for bb in dbs]
        s_dbo = load(dbo)
        impB = small.tile([128, 4 * 64], BF, tag="impB")
        nc.vector.tensor_copy(impB[:], impT[:])
        # xT[d, 64s+b] = imputed[b, 8s+d], built by strided DMAs
        xT = small.tile([D, 64 * S], BF, tag="xT")
        for d in range(D):
            for kk in range(4):
                nc.sync.dma_start(
                    out=xT[d:d + 1, 1024 * kk:1024 * (kk + 1)],
                    in_=impB[d:128:8, 64 * kk:64 * (kk + 1)])
        pools = {0: pg, 1: pt, 2: pd}
        BATCHES = ((0, 1, 2), (3, 4, 5), (6, 7))
        hprev = {}
        gts = {}
        for ci in range(5):
            hsz = DISC_SZ[ci][1]
            insz = DISC_SZ[ci][0]
            for bx, batch in enumerate(BATCHES):
                tags = {0: "g", 1: "tp", 2: "dg"}
                gt = [pools[gx].tile([96, 512], FP, tag=tags[gx],
                                     name=f"c{ci}b{bx}g{gx}")
                      for gx in range(3)]
                for bi, grp in enumerate(batch):
                    for gx in range(3):
                        if ci == 0:
                            nc.tensor.matmul(
                                gt[gx][32 * bi:32 * bi + 32, :],
                                s_w0[gx][:],
                                xT[:, 512 * grp:512 * (grp + 1)],
                                start=True, stop=True,
                                tile_position=(0, 32 * bi))
                        else:
                            sl = slice(32 * bi, 32 * bi + insz)
                            nc.tensor.matmul(
                                gt[gx][32 * bi:32 * bi + hsz, :],
                                s_dwg[ci - 1][gx][sl, :],
                                hprev[bx][sl, :],
                                start=True, stop=True,
                                tile_position=(32 * bi, 32 * bi))
                gts[bx] = gt
            for bx, batch in enumerate(BATCHES):
                gt = gts[bx]
                si = small.tile([96, 512], BF, tag=f"dsi{bx}")
                so = small.tile([96, 512], BF, tag=f"dso{bx}")
                tgd = small.tile([96, 512], BF, tag=f"dtg{bx}")
                nc.scalar.activation(si[:], gt[0][:], AF.Sigmoid,
                                     bias=s_dbs[ci][:, 0:1])
                nc.scalar.activation(so[:], gt[1][:], AF.Sigmoid,
                                     bias=s_dbs[ci][:, 1:2])
                nc.scalar.activation(tgd[:], gt[2][:], AF.Tanh,
                                     bias=s_dbs[ci][:, 2:3])
                c2 = small.tile([96, 512], BF, tag=f"dc2{bx}")
                nc.vector.tensor_mul(c2[:], si[:], tgd[:])
                nc.scalar.activation(c2[:], c2[:], AF.Tanh)
                hnew = small.tile([96, 512], BF, tag=f"dh{bx}")
                nc.vector.tensor_mul(hnew[:], so[:], c2[:])
                hprev[bx] = hnew
        for bx, batch in enumerate(BATCHES):
            for bi, grp in enumerate(batch):
                sl = slice(32 * bi, 32 * bi + 32)
                po = pg.tile([D, 512], FP, tag="g")
                nc.tensor.matmul(po[:], s_dwo3[sl, :], hprev[bx][sl, :],
                                 start=True, stop=True,
                                 tile_position=(32 * bi, 0))
                dout = small.tile([D, 512], FP, tag="dout")
                nc.scalar.activation(dout[:], po[:], AF.Identity,
                                     bias=s_dbo[:])
                nc.sync.dma_start(out=o_disc[:, 512 * grp:512 * (grp + 1)],
                                  in_=dout[:])

    nc.compile()
    return nc


# ======================= host side =======================

def _np(x):
    return np.asarray(x, dtype=np.float32)


def _reorder_gates(m, order=(1, 0, 2, 3)):
    """[i,f,g,o] blocks -> [f,i,g,o] along axis 0."""
    blocks = np.split(m, 4, axis=0)
    return np.concatenate([blocks[j] for j in order], axis=0)


def _sig(z):
    return 1.0 / (1.0 + np.exp(-np.clip(z, -60, 60)))


def _bf(x):
    return np.asarray(x, dtype=NPB)


def _ktiles(wT, n_k, width):
    """wT [n_k*128, width] -> [128, n_k*width] (tile k at cols k*width)."""
    out = np.empty((128, n_k * width), wT.dtype)
    for k in range(n_k):
        out[:, k * width:(k + 1) * width] = wT[128 * k:128 * (k + 1)]
    return out


def _lstm_host(p, x, h, c):
    g = x @ _np(p["Wih"]).T + _np(p["bih"]) + h @ _np(p["Whh"]).T + _np(p["bhh"])
    i, f, gg, o = np.split(g, 4, axis=-1)
    c2 = _sig(f) * c + _sig(i) * np.tanh(gg)
    return _sig(o) * np.tanh(c2), c2


def _prep_core_inputs(values, masks_f, params):
    flip = np.arange(H - 1, -1, -1)

    def gen_dir_weights(gp, permute_l1):
        Wih0 = _reorder_gates(_np(gp[0]["Wih"]))      # [2048, 8]
        b0 = _reorder_gates(_np(gp[0]["bih"]) + _np(gp[0]["bhh"]))
        Wih1 = _np(gp[1]["Wih"]).copy()               # [2048, 512] (in = h0)
        Whh1 = _np(gp[1]["Whh"]).copy()
        b1 = _np(gp[1]["bih"]) + _np(gp[1]["bhh"])
        if permute_l1:
            for bidx in range(4):
                sl = slice(bidx * H, (bidx + 1) * H)
                Wih1[sl] = Wih1[sl][flip]
                Whh1[sl] = Whh1[sl][flip]
                b1[sl] = b1[sl][flip]
            Whh1 = Whh1[:, flip]
        Wih1 = _reorder_gates(Wih1)
        Whh1 = _reorder_gates(Whh1)
        b1 = _reorder_gates(b1)
        Whh0 = _reorder_gates(_np(gp[0]["Whh"]))
        wih0b = np.zeros((40, GATES), np.float32)
        wih0b[0] = b0
        wih0b[32:40] = Wih0.T
        return (_bf(wih0b), _bf(_ktiles(Whh0.T, 4, GATES)),
                _bf(_ktiles(Wih1.T, 4, GATES)), _bf(_ktiles(Whh1.T, 4, GATES)),
                _bf(b1[None, :]))

    def warmup(gp, permute):
        h1v, c1v = _lstm_host(gp[1], np.zeros((1, H), np.float32),
                              np.zeros((1, H), np.float32),
                              np.zeros((1, H), np.float32))
        h1v, c1v = h1v[0], c1v[0]
        if permute:
            h1v, c1v = h1v[flip], c1v[flip]
        return h1v, c1v

    def go_weights(permute):
        W = _np(params["gen_out"]["W"]).copy()        # [8, 512]
        if permute:
            W = W[:, flip]
        return (_bf(_ktiles(np.ascontiguousarray(W.T), 4, D)),
                _bf(_np(params["gen_out"]["b"])[None, :]))

    # decoder (shared)
    dp = params["dec_cell"]
    start = np.full((1, H), 128.0, np.float32)
    hd1, cd1 = _lstm_host(dp, start, np.zeros((1, H), np.float32),
                          np.zeros((1, H), np.float32))
    Wcat = np.concatenate([_np(dp["Wih"]), _np(dp["Whh"])], axis=1)
    Wcat = _reorder_gates(Wcat)           # rows [f,i,g,o] x 512; cols [c|h]
    db = _reorder_gates(_np(dp["bih"]) + _np(dp["bhh"]))
    wdob = _bf(_np(params["dec_out"]["b"])[None, :])
    Wdo = _np(params["dec_out"]["W"])     # [8, 512]

    def dec_half(lo):
        # my gate rows: [f,i,g,o] restricted to hid [lo, lo+256)
        rows = np.concatenate([np.arange(g * H + lo, g * H + lo + 256)
                               for g in range(4)])
        Wm = Wcat[rows]                   # [1024, 1024]
        bm = db[rows]
        # state col order: c_my0,c_my1,c_par0,c_par1,h_my0,h_my1,h_par0,h_par1
        po = 256 - lo
        cols = np.concatenate([
            np.arange(lo, lo + 256), np.arange(po, po + 256),
            np.arange(512 + lo, 512 + lo + 256),
            np.arange(512 + po, 512 + po + 256)])
        WmT = np.ascontiguousarray(Wm[:, cols].T)    # [1024, 1024]
        wdecT_ = _bf(_ktiles(WmT, 8, GATES // 2))
        wdec_b_ = _bf(bm[None, :])
        dmine_ = np.concatenate(
            [cd1[0, lo:lo + 256].reshape(2, 128).T,
             hd1[0, lo:lo + 256].reshape(2, 128).T], axis=1)
        drecv_ = np.concatenate(
            [cd1[0, po:po + 256].reshape(2, 128).T,
             hd1[0, po:po + 256].reshape(2, 128).T], axis=1)
        wdoT_ = _bf(_ktiles(np.ascontiguousarray(
            np.concatenate([Wdo[:, lo:lo + 256], Wdo[:, po:po + 256]],
                           axis=1).T), 4, D))
        return (wdecT_, wdec_b_, _bf(dmine_), _bf(drecv_),
                cd1[0:1, lo:lo + 256].astype(np.float32), wdoT_)

    dec_lo = dec_half(0)
    dec_hi = dec_half(256)
    # solo-decoder (dec_split=False) arrays
    wdecT_solo = _bf(_ktiles(np.ascontiguousarray(Wcat.T), 8, GATES))
    wdecb_solo = _bf(db[None, :])
    dstate0_solo = _bf(np.concatenate(
        [cd1.reshape(4, 128).T, hd1.reshape(4, 128).T], axis=1))
    decc1_solo = cd1.reshape(1, H).astype(np.float32)
    wdoT_solo = _bf(_ktiles(np.ascontiguousarray(Wdo.T), 4, D))

    # disc (shared): pruned cells; gate blocks i/o/g
    def disc_w(i):
        p = params["disc"][i]
        W = _np(p["Wih"])
        b = _np(p["bih"]) + _np(p["bhh"])
        hsz = DISC_SZ[i][1]
        return {n: (W[bi * hsz:(bi + 1) * hsz], b[bi * hsz:(bi + 1) * hsz])
                for n, bi in (("i", 0), ("g", 2), ("o", 3))}

    b1_ = disc_w(0)
    dw0_in = [_bf(np.ascontiguousarray(b1_[name][0].T))
              for name in ("i", "o", "g")]
    dwg_in, dbs_in = [], []
    db0 = np.zeros((96, 3), np.float32)
    for bpos in range(3):
        for gx, name in enumerate(("i", "o", "g")):
            db0[32 * bpos:32 * bpos + 32, gx] = b1_[name][1]
    dbs_in.append(db0)
    for i in range(1, 5):
        blocks = disc_w(i)
        insz, hsz = DISC_SZ[i]
        row = []
        dbi = np.zeros((96, 3), np.float32)
        for gx, name in enumerate(("i", "o", "g")):
            w = np.zeros((96, hsz), np.float32)
            for bpos in range(3):
                w[32 * bpos:32 * bpos + insz] = blocks[name][0].T
                dbi[32 * bpos:32 * bpos + hsz, gx] = blocks[name][1]
            row.append(_bf(w))
        dwg_in.append(row)
        dbs_in.append(dbi)
    dwo3 = np.zeros((96, D), np.float32)
    for bpos in range(3):
        dwo3[32 * bpos:32 * bpos + 32] = _np(params["disc_out"]["W"]).T
    dwo3 = _bf(dwo3)
    dbo = _np(params["disc_out"]["b"])[:, None]

    fcWT = _bf(_ktiles(np.ascontiguousarray(_np(params["fc"]["W"]).T), 4, H))
    fcb = _bf(_np(params["fc"]["b"])[None, :])

    shared = dict(
        ident=np.eye(128, dtype=np.float32),
        ones_bf=_bf(np.ones((1, 128), np.float32)),
        ones_f=_bf(np.ones((1, 128), np.float32)),
        wdob=wdob,
        fcWT=fcWT, fcb=fcb,
        dwo3=dwo3, dbo=dbo,
        **{f"dw0_{g}": dw0_in[g] for g in range(3)},
        **{f"dw{i}_{g}": dwg_in[i - 1][g] for i in range(1, 5)
           for g in range(3)},
        **{f"db{i}": dbs_in[i] for i in range(5)},
    )

    fwd_w = gen_dir_weights(params["gen_fwd"], False)
    bwd_w = gen_dir_weights(params["gen_bwd"], True)
    fwd_h1, fwd_c1 = warmup(params["gen_fwd"], False)
    bwd_h1, bwd_c1 = warmup(params["gen_bwd"], True)
    fwd_go = go_weights(False)
    bwd_go = go_weights(True)

    in_maps = []
    for core in range(NCORES):
        is_bwd = core >= 4
        ct = core % 4
        rows = slice(128 * ct, 128 * (ct + 1))
        v = values[rows]
        m = masks_f[rows]
        if is_bwd:
            v = v[:, ::-1]
            m = m[:, ::-1]
        xq_ = (m * v).reshape(BT, S * D).astype(np.float32)
        xom_ = (1.0 - m).reshape(BT, S * D).astype(np.float32)
        (wih0b, whh0T, wih1T, whh1T, b1row) = bwd_w if is_bwd else fwd_w
        h1v, c1v = (bwd_h1, bwd_c1) if is_bwd else (fwd_h1, fwd_c1)
        goT_, gob_ = bwd_go if is_bwd else fwd_go
        h1iT = np.repeat(h1v.reshape(4, 128).T[:, :, None], BT, 2)
        h1iT = h1iT.reshape(128, 4 * BT)
        r0 = 128 * ct + (64 if is_bwd else 0)
        ve = values[r0:r0 + 64].reshape(64, 512)
        me = masks_f[r0:r0 + 64].reshape(64, 512)
        vT_ = _ktiles(np.ascontiguousarray(ve.T), 4, 64)
        mT_ = _ktiles(np.ascontiguousarray(me.T), 4, 64)
        in_maps.append(dict(
            wdecT=wdecT_solo, wdec_b=wdecb_solo, dstate0=dstate0_solo,
            dec_c1=decc1_solo, wdoT=wdoT_solo,
            xq=np.ascontiguousarray(xq_), xom=np.ascontiguousarray(xom_),
            wih0b=wih0b, whh0T=whh0T, wih1T=wih1T, whh1T=whh1T, b1row=b1row,
            goT=goT_, gob=gob_,
            h1i=np.tile(h1v, (BT, 1)).astype(np.float32),
            c1i=np.tile(c1v, (BT, 1)).astype(np.float32),
            h1iT=_bf(h1iT),
            vT=np.ascontiguousarray(vT_), mT=np.ascontiguousarray(mT_),
            **shared))
    return in_maps


_CACHE = {}


def kernel(values, masks, params):
    values = _np(values)
    masks_i = np.asarray(masks)
    masks_f = masks_i.astype(np.float32)

    if "nc" not in _CACHE:
        _CACHE["nc"] = build_program()
    nc = _CACHE["nc"]

    in_maps = _prep_core_inputs(values, masks_f, params)
    from concourse.bass_utils import run_bass_kernel_spmd
    res = run_bass_kernel_spmd(nc, in_maps, list(range(NCORES)))
    results = res.results

    impute = np.empty((B, H), np.float32)
    latent = np.empty((B, H), np.float32)
    disc = np.empty((B, S, D), np.float32)
    for core in range(NCORES):
        ct = core % 4
        r0 = 128 * ct + (64 if core >= 4 else 0)
        impute[r0:r0 + 64] = results[core]["o_imp"]
        latent[r0:r0 + 64] = results[core]["o_lat"]
        od = results[core]["o_disc"]          # [8, 4096] cols = 64*s + b
        disc[r0:r0 + 64] = od.reshape(D, S, 64).transpose(2, 1, 0)
    rec_row = results[0]["o_rec"].reshape(S, D)
    reconstructed = np.broadcast_to(rec_row, (B, S, D)).copy()
    impute = impute.reshape(B, S, D)
    return disc, impute, latent, reconstructed
